# revision 47
# baseline (speedup 1.0000x reference)
"""Trainium2 Bass kernel for nn_MultiHeadAttention_36112085025201.

Multi-head attention, B=2, S=4096, D=512, H=8 heads, Dh=64.
Sharding: 8 cores = 2 (batch) x 4 (head-pairs). Each core computes its
batch's attention for 2 heads plus that head-slice's contribution to the
output projection; the host sums the 4 partial projections per batch.

Per-core algorithm:
  - q/k/v projections in bf16 (f32 PSUM accumulate). q/k stay bf16
    ([128, S], heads stacked): scores are output-rate-bound so fp8 buys
    nothing and bf16 LDWEIGHTS are smaller. v is stored fp8e4 in vext
    slots [v(64) | ones | pad] of width 96. q/k bias is added during
    PSUM eviction on ACT (Identity + per-partition bias).
  - scores per (key-chunk t, head): bf16 matmuls [64,128]x[64,512] on
    per-head PE row groups.
  - exp is split across engines so the steady-state loop keeps the PE
    ~fully busy (the HAM clock governor re-throttles PE to 1.2 GHz
    whenever it idles; baseline measured K=4/8 for 90% of the kernel):
    head0 runs exact Exp on ACT with output *16 (bias=ln16) straight to
    fp8; head1 runs a Schraudolph approx on DVE: one tensor_scalar
    writes round(A*s+B) into an int8 view of the fp8 e-tile (bit
    pattern = fp8(~exp)). Uniform scale factors cancel per (head,
    query-block) softmax since the ones-row denominator shares them.
  - PV: fp8 DoubleRow over key-chunk PAIRS: stationary [128, 2, 96]
    (two v-slots), moving [128, 2, 512] (e pair-tile), accumulating
    [96, 512] PSUM; row 64 is the softmax denominator.
  - normalize: ACT copies the denominator row to partition 0, DVE approx
    reciprocal, GPSIMD partition broadcast, DVE multiply -> onormT bf16.
    The chains are queued and drained into the NEXT query block's steady
    state, at most ONE queued op per engine per key chunk, so they never
    stall the score pipeline (an idle PE gets clock-throttled to 1.2 GHz
    by the HAM governor, doubling every matmul; PV consumption lags
    PV_LAG chunk-pairs to give the chains WAR slack on the PSUM
    accumulators).
  - out projection in bf16 per 128-row tile pair as a dense phase at the
    END of the kernel (PE clock state no longer matters there; emitting
    it during attention starves the score-tile psum rotation and
    re-throttles the PE, even rate-limited). Evict casts to fp16 split
    ACT/DVE halves; ob pool bufs=8 so the 32 output DMAs overlap; the
    (bv@wo + bo) bias row is added on the HOST during the f32 gather --
    it is query-independent.
"""

import numpy as np
from contextlib import ExitStack

import ml_dtypes
import concourse.tile as tile
from concourse import bacc, mybir
from concourse.bass_utils import run_bass_kernel_spmd

# Problem constants (hardcoded per harness contract).
B, S, D = 2, 4096, 512
H, Dh = 8, 64
SCALE = Dh ** -0.5
N_CORES = 8
HL = 2                 # heads per core
CW = HL * Dh           # 128 local head columns per core
NK = D // 128          # 4 contraction chunks for projections
NSQ = S // 512         # 8 query blocks
NST = S // 128         # 32 key chunks (also 128-row output tiles)
NPR = NST // 2         # 16 key-chunk pairs for DoubleRow PV
VW = 96                # v slot: v dims 0..63, ones col 64, pad 65..95

BF16 = mybir.dt.bfloat16
F32 = mybir.dt.float32
F16 = mybir.dt.float16
FP8 = mybir.dt.float8e4
I8 = mybir.dt.int8
EXP = mybir.ActivationFunctionType.Exp
LN = mybir.ActivationFunctionType.Ln
IDENT = mybir.ActivationFunctionType.Identity
COPY = mybir.ActivationFunctionType.Copy
DR = mybir.MatmulPerfMode.DoubleRow

# Schraudolph constants for the DVE exp into fp8e4 bits:
# bits8 = round(A8*s + B8) ~ fp8(exp(SCALE*s) * 16 * 2^(delta/8)).
SCH_A8 = 8.0 * np.log2(np.e) * SCALE
SCH_B8 = 8.0 * (7.0 + 4.0) - 0.4
LN16 = float(np.log(16.0))


def _build_body(ctx: ExitStack, tc: "tile.TileContext", io: dict, dbg: dict | None = None):
    nc = tc.nc
    xT, wq, wk, wv, wo = io["xT"], io["wq"], io["wk"], io["wv"], io["wo"]
    bq, bk, out = io["bq"], io["bk"], io["out"]

    const = ctx.enter_context(tc.tile_pool(name="const", bufs=1))
    persist = ctx.enter_context(tc.tile_pool(name="persist", bufs=1))

    # Persistent SBUF arrays.
    xT_sb = [persist.tile([128, S], BF16, tag=f"xT{k}", name=f"xT{k}") for k in range(NK)]
    qT_sb = persist.tile([128, S], BF16, tag="qT")
    kT_sb = persist.tile([128, S], BF16, tag="kT")
    vext = [persist.tile([128, VW * NST], FP8, tag=f"vext{h}", name=f"vext{h}") for h in range(HL)]
    onormT = persist.tile([128, S], BF16, tag="onormT")

    wq_sb = [const.tile([128, CW], BF16, tag=f"wq{k}", name=f"wq{k}") for k in range(NK)]
    wk_sb = [const.tile([128, CW], BF16, tag=f"wk{k}", name=f"wk{k}") for k in range(NK)]
    wv_sb = [const.tile([128, CW], BF16, tag=f"wv{k}", name=f"wv{k}") for k in range(NK)]
    wo_sb = const.tile([128, D], BF16, tag="wo")
    bq_sb = const.tile([CW, 1], F32, tag="bq")
    bk_sb = const.tile([CW, 1], F32, tag="bk")
    lnb = const.tile([128, 1], F32, tag="lnb")

    # Input DMAs. Weights first (small, gate everything), then xT in
    # column chunks ordered the way the projections consume them.
    for k in range(NK):
        nc.sync.dma_start(wk_sb[k][:], wk[128 * k:128 * (k + 1), :])
    nc.sync.dma_start(bk_sb[:], bk[:, :])
    # All of xT next, in the order the k-projection consumes it: the k-proj
    # jp=1..3 matmuls stall on these blocks, while wv/wq are not needed
    # until the later v/q projections.
    # Quarter-chunks: a full [128,1024] chunk is 256KB on ONE dma queue
    # (~12us at ~22GB/s) and gated the first k-proj matmul; four 64KB
    # pieces on four queues land the first moving operand in ~3us.
    for jp in range(NSQ // 2):
        for k in range(NK):
            for q in range(4):
                c0 = 1024 * jp + 256 * q
                nc.sync.dma_start(xT_sb[k][:, c0:c0 + 256],
                                  xT[128 * k:128 * (k + 1), c0:c0 + 256])
    for k in range(NK):
        nc.sync.dma_start(wv_sb[k][:], wv[128 * k:128 * (k + 1), :])
        nc.sync.dma_start(wq_sb[k][:], wq[128 * k:128 * (k + 1), :])
    nc.sync.dma_start(bq_sb[:], bq[:, :])
    nc.sync.dma_start(wo_sb[:], wo[:, :])

    # PSUM pools (8 banks): pmm 2x[128,1024] = 4, pacc 4x[96,512] = 4.
    pmm = ctx.enter_context(tc.tile_pool(name="pmm", bufs=2, space="PSUM"))
    pacc = ctx.enter_context(tc.tile_pool(name="pacc", bufs=1, space="PSUM"))

    nc.vector.memset(lnb[:], LN16)

    expp = ctx.enter_context(tc.tile_pool(name="expp", bufs=10))
    rp = ctx.enter_context(tc.tile_pool(name="rp", bufs=4))
    outp = ctx.enter_context(tc.tile_pool(name="outp", bufs=8))

    # Phase A/B: projections (bf16 matmuls, bf16 q/k: fp8 gives scores no
    # PE benefit -- they are output-rate-bound -- and bf16 LDWEIGHTS are
    # smaller and the accuracy better).
    def qk_proj(w_sb, b_sb, dst):
        for jp in range(NSQ // 2):
            ps = pmm.tile([128, 1024], F32, tag="mm")
            for k in range(NK):
                for jj in range(2):
                    nc.tensor.matmul(ps[:, 512 * jj:512 * (jj + 1)], w_sb[k][:],
                                     xT_sb[k][:, 1024 * jp + 512 * jj:1024 * jp + 512 * (jj + 1)],
                                     start=(k == 0), stop=(k == NK - 1))
            # Bias-add on ACT (Identity + per-partition bias): keeps DVE free
            # for the Schraudolph half of the exp work in phase C.
            nc.scalar.activation(dst[:, 1024 * jp:1024 * (jp + 1)], ps[:],
                                 IDENT, bias=b_sb[:])

    qk_proj(wk_sb, bk_sb, kT_sb)
    qk_proj(wq_sb, bq_sb, qT_sb)

    # v projection in normal orientation [s, c]; fp8 eviction into vext
    # slots, ones column per chunk at slot col 64. The pad columns
    # (65..95) stay UNinitialized: they only feed PV output partitions
    # 65..95, which are never read. Evictions are one strided
    # two-chunk copy per head, split DVE/ACT, so neither engine gates
    # the psum-tile rotation (a gated PE cools the HAM clock).
    for h in range(HL):
        nc.vector.memset(
            vext[h][:].rearrange("p (t w) -> p t w", w=VW)[:, :, Dh:Dh + 1], 1.0)
    for tp in range(NST // 2):
        ps = pmm.tile([128, 1024], F32, tag="mm")
        for tt in range(2):
            t = 2 * tp + tt
            for k in range(NK):
                nc.tensor.matmul(ps[:, 512 * tt:512 * tt + CW],
                                 xT_sb[k][:, 128 * t:128 * (t + 1)], wv_sb[k][:],
                                 start=(k == 0), stop=(k == NK - 1))
        for h in range(HL):
            src = ps[:].rearrange("p (tt c) -> p tt c", tt=2)[:, :, Dh * h:Dh * (h + 1)]
            dst = (vext[h][:, VW * 2 * tp:VW * 2 * (tp + 1)]
                   .rearrange("p (t w) -> p t w", w=VW)[:, :, 0:Dh])
            if h == 0:
                nc.vector.tensor_copy(dst, src)
            else:
                nc.scalar.activation(dst, src, COPY)

    # Phase C: streaming attention (fp8 DoubleRow).
    #
    # The HAM clock governor re-throttles the PE to 1.2 GHz after idle
    # windows, which doubles every matmul. The v2 profile showed K=4/8
    # interludes at every query-block boundary: the normalization +
    # out-projection work emitted there stalled the score pipeline (the
    # out-proj matmuls depend on the multi-us normalization chains and
    # head-of-line-blocked the PE; their psum tiles also injected that
    # latency into the score-tile rotation). So: normalization chains are
    # queued as closures and DRAINED two per key chunk into the next
    # block's steady state (no PE content -> no head-of-line risk), the
    # out-projection runs as a dense phase D after the key loop, and PV
    # consumption lags by PV_LAG chunk-pairs so the normalization deadline
    # (WAR on the PSUM accumulators) has slack.
    #
    # The drain is ENGINE-AWARE: at most one queued op per engine per key
    # chunk. (Draining a recip+mul pair put two DVE ops between
    # consecutive buf-freeing Schraudolphs, stretching the period until
    # the MID throttler fired -- the 6-14us K=4/8 dips in the v5 profile.)
    PV_LAG = 4

    bqueue = []  # (engine, closure) in dependency-safe FIFO order
    all_pieces = []

    def drain():
        used = set()
        while bqueue and bqueue[0][0] not in used:
            eng, fn = bqueue.pop(0)
            fn()
            used.add(eng)

    for jp in range(NSQ // 2):
        j0 = 2 * jp
        po = {(h, jj): pacc.tile([VW, 512], F32, tag=f"acc{h}{jj}", name=f"po{h}{jj}")
              for h in range(HL) for jj in range(2)}

        # PV for key-chunk pair pr-PV_LAG is emitted one HEAD at a time:
        # h0's two matmuls after the even chunk's exp, h1's after the odd
        # chunk's, keeping PE work spread between a chunk's last score
        # matmul and the next chunk's reuse of its PSUM slot.
        def emit_pv_head(e_hist, pr_prev, h, po=po):
            for jj in range(2):
                nc.tensor.matmul(
                    po[(h, jj)][:],
                    vext[h][:, 2 * VW * pr_prev:2 * VW * (pr_prev + 1)]
                        .rearrange("p (k m) -> p k m", k=2),
                    e_hist[pr_prev][h][:].rearrange("p (k n) -> p k n", k=2)
                        [:, :, 512 * jj:512 * (jj + 1)],
                    start=(pr_prev == 0), stop=(pr_prev == NPR - 1),
                    perf_mode=DR)

        e_hist = []
        for pr in range(NPR):
            e_hist.append({h: expp.tile([128, 2048], FP8, tag="e", name=f"e{h}")
                           for h in range(HL)})
            e_cur = e_hist[pr]
            for tt in range(2):
                t = 2 * pr + tt
                s = {}
                for h in range(HL):
                    s[h] = pmm.tile([128, 1024], F32, tag="mm", name=f"s{h}")
                    for jj in range(2):
                        nc.tensor.matmul(
                            s[h][:, 512 * jj:512 * (jj + 1)],
                            kT_sb[Dh * h:Dh * (h + 1), 128 * t:128 * (t + 1)],
                            qT_sb[Dh * h:Dh * (h + 1),
                                  512 * (j0 + jj):512 * (j0 + jj + 1)],
                            start=True, stop=True)
                # Exp split by head so ACT and DVE each carry half: h0 runs
                # exact Exp on ACT (x16 centering); h1 runs a Schraudolph
                # approx on DVE -- round(A8*s+B8) into an int8 view of the
                # fp8 e-tile. Each head's softmax is served by one engine
                # for all key chunks, so the Schraudolph uniform scale error
                # cancels against that head's ones-row denominator.
                off = 1024 * tt
                nc.scalar.activation(e_cur[0][:, off:off + 1024], s[0][:],
                                     EXP, bias=lnb[:], scale=float(SCALE))
                nc.vector.tensor_scalar(e_cur[1][:, off:off + 1024].bitcast(I8),
                                        s[1][:], SCH_A8, SCH_B8,
                                        op0=mybir.AluOpType.mult,
                                        op1=mybir.AluOpType.add)
                if pr >= PV_LAG:
                    emit_pv_head(e_hist, pr - PV_LAG, tt)
                drain()
            if pr == NPR - 1:
                # Tail of the lagged PV pipeline: dense PE work, keeps the
                # PE warm across the block boundary.
                for p in range(NPR - PV_LAG, NPR):
                    for h in range(HL):
                        emit_pv_head(e_hist, p, h)

        # Queue this block's normalization; jp+1's steady state drains it.
        # The muls must land before jp+1's first PV (start=True) reuses
        # the accumulators -- PV_LAG covers that.
        def enqueue_norm(h, jj, po=po, j0=j0):
            j = j0 + jj
            st = {}

            def c_():
                # NB: Ln+Exp(-x) would avoid the DVE recip, but Ln lives in
                # a different ACT table set than Exp here, and the table
                # loader thrashes (31 ACT_TABLE_LOADs, +40us). Keep the
                # ACT copy (custom-DVE ucode recip needs base partition 0)
                # + DVE approx reciprocal.
                st["r0"] = rp.tile([1, 512], F32, tag="r0", name="r0")
                nc.scalar.activation(st["r0"][:], po[(h, jj)][Dh:Dh + 1, :], COPY)

            def r_():
                st["r"] = rp.tile([1, 512], F32, tag="r", name="r")
                nc.vector.reciprocal_approx_fast(st["r"][:], st["r0"][:])

            def b_():
                st["rb"] = rp.tile([Dh, 512], F32, tag="rb", name="rb")
                nc.gpsimd.partition_broadcast(st["rb"][:], st["r"][:])

            def m_():
                nc.vector.tensor_mul(
                    onormT[Dh * h:Dh * (h + 1), 512 * j:512 * (j + 1)],
                    po[(h, jj)][0:Dh, :], st["rb"][:])

            bqueue.extend([("A", c_), ("D", r_), ("G", b_), ("D", m_)])

        def out_proj_piece(sp, act_only, jp_=jp):
            # One pair of 128-row query tiles: one [128,1024] psum, two
            # matmuls, two output DMAs. Early pieces evict on ACT only:
            # the DVE is still draining the last block's norm chains, and
            # a DVE-evicted piece stalls the 2-buf pf rotation (and the
            # PE behind it).
            sq0 = 1024 * jp_ + 256 * sp
            pf = pmm.tile([128, 1024], F32, tag="mm", name="pf")
            for u in range(2):
                nc.tensor.matmul(pf[:, 512 * u:512 * (u + 1)],
                                 onormT[:, sq0 + 128 * u:sq0 + 128 * (u + 1)],
                                 wo_sb[:], start=True, stop=True)
            ob = outp.tile([128, 1024], F16, tag="ob")
            if act_only:
                nc.scalar.activation(ob[:], pf[:], COPY)
            else:
                nc.scalar.activation(ob[:, 0:512], pf[:, 0:512], COPY)
                nc.vector.tensor_copy(ob[:, 512:1024], pf[:, 512:1024])
            for u in range(2):
                nc.sync.dma_start(out[sq0 + 128 * u:sq0 + 128 * (u + 1), :],
                                  ob[:, 512 * u:512 * (u + 1)])

        for jj in range(2):
            for h in range(HL):
                enqueue_norm(h, jj)
        all_pieces.append(out_proj_piece)

    while bqueue:
        bqueue.pop(0)[1]()

    # Phase D: out projection, dense at the end. Mid-attention emission
    # was tried twice (v4 lump, v8 rate-limited drain): both cool the PE
    # via the pf psum-tile injection into the score rotation. The output
    # DMA needs many ob bufs in flight (outp bufs=8) or the ~6us/128KB
    # per-queue transfers serialize the whole phase.
    for i, piece in enumerate(all_pieces):
        for sp in range(4):
            piece(sp, act_only=(i < 2))

    if dbg:
        for name, sb in (("onormT", onormT), ("vext0", vext[0]), ("vext1", vext[1]),
                         ("qT", qT_sb), ("kT", kT_sb)):
            if name in dbg:
                nc.sync.dma_start(dbg[name][:, :], sb[:])


def build_nc():
    nc = bacc.Bacc("TRN2", target_bir_lowering=False, debug=False,
                   enable_asserts=False, num_devices=N_CORES)
    io = {
        "xT": nc.dram_tensor("xT", [D, S], BF16, kind="ExternalInput").ap(),
        "wq": nc.dram_tensor("wq", [D, CW], BF16, kind="ExternalInput").ap(),
        "wk": nc.dram_tensor("wk", [D, CW], BF16, kind="ExternalInput").ap(),
        "wv": nc.dram_tensor("wv", [D, CW], BF16, kind="ExternalInput").ap(),
        "wo": nc.dram_tensor("wo", [CW, D], BF16, kind="ExternalInput").ap(),
        "bq": nc.dram_tensor("bq", [CW, 1], F32, kind="ExternalInput").ap(),
        "bk": nc.dram_tensor("bk", [CW, 1], F32, kind="ExternalInput").ap(),
        "out": nc.dram_tensor("out", [S, D], F16, kind="ExternalOutput").ap(),
    }
    with tile.TileContext(nc) as tc, ExitStack() as ctx:
        _build_body(ctx, tc, io)
    nc.compile()
    return nc


def make_in_maps(x, wq, bq, wk, bk, wv, bv, wo, bo):
    """Shard the full inputs across the 8 cores (host-side marshalling)."""
    bf16 = ml_dtypes.bfloat16
    in_maps = []
    for c in range(N_CORES):
        b, hp = divmod(c, 4)
        cs = slice(CW * hp, CW * (hp + 1))
        xT = np.ascontiguousarray(x[b].T).astype(bf16)
        in_maps.append({
            "xT": xT,
            "wq": np.ascontiguousarray(wq[:, cs]).astype(bf16),
            "wk": np.ascontiguousarray(wk[:, cs]).astype(bf16),
            "wv": np.ascontiguousarray(wv[:, cs]).astype(bf16),
            "wo": np.ascontiguousarray(wo[cs, :]).astype(bf16),
            "bq": np.ascontiguousarray(bq[cs].reshape(CW, 1)).astype(np.float32),
            "bk": np.ascontiguousarray(bk[cs].reshape(CW, 1)).astype(np.float32),
        })
    return in_maps


_CACHE = {}


def _get_nc():
    if "nc" not in _CACHE:
        _CACHE["nc"] = build_nc()
    return _CACHE["nc"]


def run_sharded(nc, in_maps, **kwargs):
    return run_bass_kernel_spmd(nc, in_maps, core_ids=list(range(N_CORES)), **kwargs)


def gather(results, bvwo):
    # The query-independent output bias row (bv@wo + bo) is added here on
    # the host: it would cost a DVE tensor_tensor per output tile on-device.
    out = np.zeros((B, S, D), np.float32)
    for c in range(N_CORES):
        out[c // 4] += results[c]["out"].astype(np.float32)
    out += bvwo.reshape(1, 1, D)
    return out


def host_bias(bv, wo, bo):
    return (bv.astype(np.float64) @ wo.astype(np.float64)
            + bo.astype(np.float64)).astype(np.float32)


def kernel(x, wq, bq, wk, bk, wv, bv, wo, bo):
    x, wq, bq, wk, bk, wv, bv, wo, bo = (
        np.asarray(a, np.float32) for a in (x, wq, bq, wk, bk, wv, bv, wo, bo))
    nc = _get_nc()
    in_maps = make_in_maps(x, wq, bq, wk, bk, wv, bv, wo, bo)
    res = run_sharded(nc, in_maps)
    return gather(res.results, host_bias(bv, wo, bo))



# revision 49
# speedup vs baseline: 1.0822x; 1.0822x over previous
"""Trainium2 Bass kernel for nn_MultiHeadAttention_36112085025201.

Multi-head attention, B=2, S=4096, D=512, H=8 heads, Dh=64.
Sharding: 8 cores = 2 (batch) x 4 (head-pairs). Each core computes its
batch's attention for 2 heads plus that head-slice's contribution to the
output projection; the host sums the 4 partial projections per batch.

Per-core algorithm:
  - q/k/v projections in bf16 (f32 PSUM accumulate). q/k stay bf16
    ([128, S], heads stacked): scores are output-rate-bound so fp8 buys
    nothing and bf16 LDWEIGHTS are smaller. v is stored fp8e4 in vext
    slots [v(64) | ones | pad] of width 96. q/k bias is added during
    PSUM eviction on ACT (Identity + per-partition bias).
  - scores per (key-chunk t, head): bf16 matmuls [64,128]x[64,512] on
    per-head PE row groups.
  - exp is split across engines so the steady-state loop keeps the PE
    ~fully busy (the HAM clock governor re-throttles PE to 1.2 GHz
    whenever it idles; baseline measured K=4/8 for 90% of the kernel):
    head0 runs exact Exp on ACT with output *16 (bias=ln16) straight to
    fp8; head1 runs a Schraudolph approx on DVE: one tensor_scalar
    writes round(A*s+B) into an int8 view of the fp8 e-tile (bit
    pattern = fp8(~exp)). Uniform scale factors cancel per (head,
    query-block) softmax since the ones-row denominator shares them.
  - PV: fp8 DoubleRow over key-chunk PAIRS: stationary [128, 2, 96]
    (two v-slots), moving [128, 2, 512] (e pair-tile), accumulating
    [96, 512] PSUM; row 64 is the softmax denominator.
  - normalize: ACT copies the denominator row to partition 0, DVE approx
    reciprocal, GPSIMD partition broadcast, DVE multiply -> onormT bf16.
    The chains are queued and drained into the NEXT query block's steady
    state, at most ONE queued op per engine per key chunk, so they never
    stall the score pipeline (an idle PE gets clock-throttled to 1.2 GHz
    by the HAM governor, doubling every matmul; PV consumption lags
    PV_LAG chunk-pairs to give the chains WAR slack on the PSUM
    accumulators).
  - out projection in bf16 per 128-row tile pair as a dense phase at the
    END of the kernel (PE clock state no longer matters there; emitting
    it during attention starves the score-tile psum rotation and
    re-throttles the PE, even rate-limited). Evict casts to fp16 split
    ACT/DVE halves; ob pool bufs=8 so the 32 output DMAs overlap; the
    (bv@wo + bo) bias row is added on the HOST during the f32 gather --
    it is query-independent.
"""

import numpy as np
from contextlib import ExitStack

import ml_dtypes
import concourse.tile as tile
from concourse import bacc, mybir
from concourse.bass_utils import run_bass_kernel_spmd

# Problem constants (hardcoded per harness contract).
B, S, D = 2, 4096, 512
H, Dh = 8, 64
SCALE = Dh ** -0.5
N_CORES = 8
HL = 2                 # heads per core
CW = HL * Dh           # 128 local head columns per core
NK = D // 128          # 4 contraction chunks for projections
NSQ = S // 512         # 8 query blocks
NST = S // 128         # 32 key chunks (also 128-row output tiles)
NPR = NST // 2         # 16 key-chunk pairs for DoubleRow PV
VW = 80                # v slot: v dims 0..63, ones col 64, pad 65..79

BF16 = mybir.dt.bfloat16
F32 = mybir.dt.float32
F16 = mybir.dt.float16
FP8 = mybir.dt.float8e4
I8 = mybir.dt.int8
EXP = mybir.ActivationFunctionType.Exp
LN = mybir.ActivationFunctionType.Ln
IDENT = mybir.ActivationFunctionType.Identity
COPY = mybir.ActivationFunctionType.Copy
DR = mybir.MatmulPerfMode.DoubleRow

# Schraudolph constants for the DVE exp into fp8e4 bits:
# bits8 = round(A8*s + B8) ~ fp8(exp(SCALE*s) * 16 * 2^(delta/8)).
SCH_A8 = 8.0 * np.log2(np.e) * SCALE
SCH_B8 = 8.0 * (7.0 + 4.0) - 0.4
LN16 = float(np.log(16.0))


def _build_body(ctx: ExitStack, tc: "tile.TileContext", io: dict, dbg: dict | None = None):
    nc = tc.nc
    xT, wq, wk, wv, wo = io["xT"], io["wq"], io["wk"], io["wv"], io["wo"]
    bq, bk, out = io["bq"], io["bk"], io["out"]

    const = ctx.enter_context(tc.tile_pool(name="const", bufs=1))
    persist = ctx.enter_context(tc.tile_pool(name="persist", bufs=1))

    # Persistent SBUF arrays.
    xT_sb = [persist.tile([128, S], BF16, tag=f"xT{k}", name=f"xT{k}") for k in range(NK)]
    qT_sb = persist.tile([128, S], BF16, tag="qT")
    kT_sb = persist.tile([128, S], BF16, tag="kT")
    vext = [persist.tile([128, VW * NST], FP8, tag=f"vext{h}", name=f"vext{h}") for h in range(HL)]
    onormT = persist.tile([128, S], BF16, tag="onormT")

    wq_sb = [const.tile([128, CW], BF16, tag=f"wq{k}", name=f"wq{k}") for k in range(NK)]
    wk_sb = [const.tile([128, CW], BF16, tag=f"wk{k}", name=f"wk{k}") for k in range(NK)]
    wv_sb = [const.tile([128, CW], BF16, tag=f"wv{k}", name=f"wv{k}") for k in range(NK)]
    wo_sb = const.tile([128, D], BF16, tag="wo")
    bq_sb = const.tile([CW, 1], F32, tag="bq")
    bk_sb = const.tile([CW, 1], F32, tag="bk")
    lnb = const.tile([128, 1], F32, tag="lnb")

    # Input DMAs. Weights first (small, gate everything), then xT in
    # column chunks ordered the way the projections consume them.
    for k in range(NK):
        nc.sync.dma_start(wk_sb[k][:], wk[128 * k:128 * (k + 1), :])
    nc.sync.dma_start(bk_sb[:], bk[:, :])
    # All of xT next, in the order the k-projection consumes it: the k-proj
    # jp=1..3 matmuls stall on these blocks, while wv/wq are not needed
    # until the later v/q projections.
    for jp in range(NSQ // 2):
        for k in range(NK):
            nc.sync.dma_start(xT_sb[k][:, 1024 * jp:1024 * (jp + 1)],
                              xT[128 * k:128 * (k + 1), 1024 * jp:1024 * (jp + 1)])
    for k in range(NK):
        nc.sync.dma_start(wv_sb[k][:], wv[128 * k:128 * (k + 1), :])
        nc.sync.dma_start(wq_sb[k][:], wq[128 * k:128 * (k + 1), :])
    nc.sync.dma_start(bq_sb[:], bq[:, :])
    nc.sync.dma_start(wo_sb[:], wo[:, :])

    # PSUM pools (8 banks): pmm 2x[128,1024] = 4, pacc 4x[96,512] = 4.
    pmm = ctx.enter_context(tc.tile_pool(name="pmm", bufs=2, space="PSUM"))
    pacc = ctx.enter_context(tc.tile_pool(name="pacc", bufs=1, space="PSUM"))

    nc.vector.memset(lnb[:], LN16)

    expp = ctx.enter_context(tc.tile_pool(name="expp", bufs=10))
    rp = ctx.enter_context(tc.tile_pool(name="rp", bufs=4))
    outp = ctx.enter_context(tc.tile_pool(name="outp", bufs=8))

    # Phase A/B: projections (bf16 matmuls, bf16 q/k: fp8 gives scores no
    # PE benefit -- they are output-rate-bound -- and bf16 LDWEIGHTS are
    # smaller and the accuracy better).
    def qk_proj(w_sb, b_sb, dst):
        for jp in range(NSQ // 2):
            ps = pmm.tile([128, 1024], F32, tag="mm")
            for k in range(NK):
                for jj in range(2):
                    nc.tensor.matmul(ps[:, 512 * jj:512 * (jj + 1)], w_sb[k][:],
                                     xT_sb[k][:, 1024 * jp + 512 * jj:1024 * jp + 512 * (jj + 1)],
                                     start=(k == 0), stop=(k == NK - 1))
            # Bias-add on ACT (Identity + per-partition bias): keeps DVE free
            # for the Schraudolph half of the exp work in phase C.
            nc.scalar.activation(dst[:, 1024 * jp:1024 * (jp + 1)], ps[:],
                                 IDENT, bias=b_sb[:])

    qk_proj(wk_sb, bk_sb, kT_sb)
    qk_proj(wq_sb, bq_sb, qT_sb)

    # v projection in normal orientation [s, c]; fp8 eviction into vext
    # slots, ones column per chunk at slot col 64. The pad columns
    # (65..95) stay UNinitialized: they only feed PV output partitions
    # 65..95, which are never read. Evictions are one strided
    # two-chunk copy per head, split DVE/ACT, so neither engine gates
    # the psum-tile rotation (a gated PE cools the HAM clock).
    for h in range(HL):
        nc.vector.memset(
            vext[h][:].rearrange("p (t w) -> p t w", w=VW)[:, :, Dh:Dh + 1], 1.0)
    for tp in range(NST // 2):
        ps = pmm.tile([128, 1024], F32, tag="mm")
        for tt in range(2):
            t = 2 * tp + tt
            for k in range(NK):
                nc.tensor.matmul(ps[:, 512 * tt:512 * tt + CW],
                                 xT_sb[k][:, 128 * t:128 * (t + 1)], wv_sb[k][:],
                                 start=(k == 0), stop=(k == NK - 1))
        for h in range(HL):
            src = ps[:].rearrange("p (tt c) -> p tt c", tt=2)[:, :, Dh * h:Dh * (h + 1)]
            dst = (vext[h][:, VW * 2 * tp:VW * 2 * (tp + 1)]
                   .rearrange("p (t w) -> p t w", w=VW)[:, :, 0:Dh])
            if h == 0:
                nc.vector.tensor_copy(dst, src)
            else:
                nc.scalar.activation(dst, src, COPY)

    # Phase C: streaming attention (fp8 DoubleRow).
    #
    # The HAM clock governor re-throttles the PE to 1.2 GHz after idle
    # windows, which doubles every matmul. The v2 profile showed K=4/8
    # interludes at every query-block boundary: the normalization +
    # out-projection work emitted there stalled the score pipeline (the
    # out-proj matmuls depend on the multi-us normalization chains and
    # head-of-line-blocked the PE; their psum tiles also injected that
    # latency into the score-tile rotation). So: normalization chains are
    # queued as closures and DRAINED two per key chunk into the next
    # block's steady state (no PE content -> no head-of-line risk), the
    # out-projection runs as a dense phase D after the key loop, and PV
    # consumption lags by PV_LAG chunk-pairs so the normalization deadline
    # (WAR on the PSUM accumulators) has slack.
    #
    # The drain is ENGINE-AWARE: at most one queued op per engine per key
    # chunk. (Draining a recip+mul pair put two DVE ops between
    # consecutive buf-freeing Schraudolphs, stretching the period until
    # the MID throttler fired -- the 6-14us K=4/8 dips in the v5 profile.)
    PV_LAG = 4

    bqueue = []  # (engine, closure) in dependency-safe FIFO order
    all_pieces = []

    def drain():
        used = set()
        while bqueue and bqueue[0][0] not in used:
            eng, fn = bqueue.pop(0)
            fn()
            used.add(eng)

    for jp in range(NSQ // 2):
        j0 = 2 * jp
        po = {(h, jj): pacc.tile([VW, 512], F32, tag=f"acc{h}{jj}", name=f"po{h}{jj}")
              for h in range(HL) for jj in range(2)}

        # PV for key-chunk pair pr-PV_LAG is emitted one HEAD at a time:
        # h0's two matmuls after the even chunk's exp, h1's after the odd
        # chunk's, keeping PE work spread between a chunk's last score
        # matmul and the next chunk's reuse of its PSUM slot.
        def emit_pv_head(e_hist, pr_prev, h, po=po):
            for jj in range(2):
                nc.tensor.matmul(
                    po[(h, jj)][:],
                    vext[h][:, 2 * VW * pr_prev:2 * VW * (pr_prev + 1)]
                        .rearrange("p (k m) -> p k m", k=2),
                    e_hist[pr_prev][h][:].rearrange("p (k n) -> p k n", k=2)
                        [:, :, 512 * jj:512 * (jj + 1)],
                    start=(pr_prev == 0), stop=(pr_prev == NPR - 1),
                    perf_mode=DR)

        e_hist = []
        for pr in range(NPR):
            e_hist.append({h: expp.tile([128, 2048], FP8, tag="e", name=f"e{h}")
                           for h in range(HL)})
            e_cur = e_hist[pr]
            for tt in range(2):
                t = 2 * pr + tt
                s = {}
                for h in range(HL):
                    s[h] = pmm.tile([128, 1024], F32, tag="mm", name=f"s{h}")
                    for jj in range(2):
                        nc.tensor.matmul(
                            s[h][:, 512 * jj:512 * (jj + 1)],
                            kT_sb[Dh * h:Dh * (h + 1), 128 * t:128 * (t + 1)],
                            qT_sb[Dh * h:Dh * (h + 1),
                                  512 * (j0 + jj):512 * (j0 + jj + 1)],
                            start=True, stop=True)
                # Exp split by head so ACT and DVE each carry half: h0 runs
                # exact Exp on ACT (x16 centering); h1 runs a Schraudolph
                # approx on DVE -- round(A8*s+B8) into an int8 view of the
                # fp8 e-tile. Each head's softmax is served by one engine
                # for all key chunks, so the Schraudolph uniform scale error
                # cancels against that head's ones-row denominator.
                off = 1024 * tt
                nc.scalar.activation(e_cur[0][:, off:off + 1024], s[0][:],
                                     EXP, bias=lnb[:], scale=float(SCALE))
                nc.vector.tensor_scalar(e_cur[1][:, off:off + 1024].bitcast(I8),
                                        s[1][:], SCH_A8, SCH_B8,
                                        op0=mybir.AluOpType.mult,
                                        op1=mybir.AluOpType.add)
                if pr >= PV_LAG:
                    emit_pv_head(e_hist, pr - PV_LAG, tt)
                drain()
            if pr == NPR - 1:
                # Tail of the lagged PV pipeline: dense PE work, keeps the
                # PE warm across the block boundary.
                for p in range(NPR - PV_LAG, NPR):
                    for h in range(HL):
                        emit_pv_head(e_hist, p, h)

        # Queue this block's normalization; jp+1's steady state drains it.
        # The muls must land before jp+1's first PV (start=True) reuses
        # the accumulators -- PV_LAG covers that.
        def enqueue_norm(h, jj, po=po, j0=j0):
            j = j0 + jj
            st = {}

            def c_():
                # NB: Ln+Exp(-x) would avoid the DVE recip, but Ln lives in
                # a different ACT table set than Exp here, and the table
                # loader thrashes (31 ACT_TABLE_LOADs, +40us). Keep the
                # ACT copy (custom-DVE ucode recip needs base partition 0)
                # + DVE approx reciprocal.
                st["r0"] = rp.tile([1, 512], F32, tag="r0", name="r0")
                nc.scalar.activation(st["r0"][:], po[(h, jj)][Dh:Dh + 1, :], COPY)

            def r_():
                st["r"] = rp.tile([1, 512], F32, tag="r", name="r")
                nc.vector.reciprocal_approx_fast(st["r"][:], st["r0"][:])

            def b_():
                st["rb"] = rp.tile([Dh, 512], F32, tag="rb", name="rb")
                nc.gpsimd.partition_broadcast(st["rb"][:], st["r"][:])

            def m_():
                nc.vector.tensor_mul(
                    onormT[Dh * h:Dh * (h + 1), 512 * j:512 * (j + 1)],
                    po[(h, jj)][0:Dh, :], st["rb"][:])

            bqueue.extend([("A", c_), ("D", r_), ("G", b_), ("D", m_)])

        def out_proj_piece(sp, act_only, jp_=jp):
            # One pair of 128-row query tiles: one [128,1024] psum, two
            # matmuls, evict halves split ACT/DVE, two output DMAs.
            sq0 = 1024 * jp_ + 256 * sp
            pf = pmm.tile([128, 1024], F32, tag="mm", name="pf")
            for u in range(2):
                nc.tensor.matmul(pf[:, 512 * u:512 * (u + 1)],
                                 onormT[:, sq0 + 128 * u:sq0 + 128 * (u + 1)],
                                 wo_sb[:], start=True, stop=True)
            ob = outp.tile([128, 1024], F16, tag="ob")
            if act_only:
                nc.scalar.activation(ob[:], pf[:], COPY)
            else:
                nc.scalar.activation(ob[:, 0:512], pf[:, 0:512], COPY)
                nc.vector.tensor_copy(ob[:, 512:1024], pf[:, 512:1024])
            for u in range(2):
                nc.sync.dma_start(out[sq0 + 128 * u:sq0 + 128 * (u + 1), :],
                                  ob[:, 512 * u:512 * (u + 1)])

        for jj in range(2):
            for h in range(HL):
                enqueue_norm(h, jj)
        all_pieces.append(out_proj_piece)

    while bqueue:
        bqueue.pop(0)[1]()

    # Phase D: out projection, dense at the end. Mid-attention emission
    # was tried twice (v4 lump, v8 rate-limited drain): both cool the PE
    # via the pf psum-tile injection into the score rotation. The output
    # DMA needs many ob bufs in flight (outp bufs=8) or the ~6us/128KB
    # per-queue transfers serialize the whole phase.
    for i, piece in enumerate(all_pieces):
        for sp in range(4):
            piece(sp, act_only=(i < 2))

    if dbg:
        for name, sb in (("onormT", onormT), ("vext0", vext[0]), ("vext1", vext[1]),
                         ("qT", qT_sb), ("kT", kT_sb)):
            if name in dbg:
                nc.sync.dma_start(dbg[name][:, :], sb[:])


def build_nc():
    nc = bacc.Bacc("TRN2", target_bir_lowering=False, debug=False,
                   enable_asserts=False, num_devices=N_CORES)
    io = {
        "xT": nc.dram_tensor("xT", [D, S], BF16, kind="ExternalInput").ap(),
        "wq": nc.dram_tensor("wq", [D, CW], BF16, kind="ExternalInput").ap(),
        "wk": nc.dram_tensor("wk", [D, CW], BF16, kind="ExternalInput").ap(),
        "wv": nc.dram_tensor("wv", [D, CW], BF16, kind="ExternalInput").ap(),
        "wo": nc.dram_tensor("wo", [CW, D], BF16, kind="ExternalInput").ap(),
        "bq": nc.dram_tensor("bq", [CW, 1], F32, kind="ExternalInput").ap(),
        "bk": nc.dram_tensor("bk", [CW, 1], F32, kind="ExternalInput").ap(),
        "out": nc.dram_tensor("out", [S, D], F16, kind="ExternalOutput").ap(),
    }
    with tile.TileContext(nc) as tc, ExitStack() as ctx:
        _build_body(ctx, tc, io)
    nc.compile()
    return nc


def make_in_maps(x, wq, bq, wk, bk, wv, bv, wo, bo):
    """Shard the full inputs across the 8 cores (host-side marshalling)."""
    bf16 = ml_dtypes.bfloat16
    in_maps = []
    for c in range(N_CORES):
        b, hp = divmod(c, 4)
        cs = slice(CW * hp, CW * (hp + 1))
        xT = np.ascontiguousarray(x[b].T).astype(bf16)
        in_maps.append({
            "xT": xT,
            "wq": np.ascontiguousarray(wq[:, cs]).astype(bf16),
            "wk": np.ascontiguousarray(wk[:, cs]).astype(bf16),
            "wv": np.ascontiguousarray(wv[:, cs]).astype(bf16),
            "wo": np.ascontiguousarray(wo[cs, :]).astype(bf16),
            "bq": np.ascontiguousarray(bq[cs].reshape(CW, 1)).astype(np.float32),
            "bk": np.ascontiguousarray(bk[cs].reshape(CW, 1)).astype(np.float32),
        })
    return in_maps


_CACHE = {}


def _get_nc():
    if "nc" not in _CACHE:
        _CACHE["nc"] = build_nc()
    return _CACHE["nc"]


def run_sharded(nc, in_maps, **kwargs):
    return run_bass_kernel_spmd(nc, in_maps, core_ids=list(range(N_CORES)), **kwargs)


def gather(results, bvwo):
    # The query-independent output bias row (bv@wo + bo) is added here on
    # the host: it would cost a DVE tensor_tensor per output tile on-device.
    out = np.zeros((B, S, D), np.float32)
    for c in range(N_CORES):
        out[c // 4] += results[c]["out"].astype(np.float32)
    out += bvwo.reshape(1, 1, D)
    return out


def host_bias(bv, wo, bo):
    return (bv.astype(np.float64) @ wo.astype(np.float64)
            + bo.astype(np.float64)).astype(np.float32)


def kernel(x, wq, bq, wk, bk, wv, bv, wo, bo):
    x, wq, bq, wk, bk, wv, bv, wo, bo = (
        np.asarray(a, np.float32) for a in (x, wq, bq, wk, bk, wv, bv, wo, bo))
    nc = _get_nc()
    in_maps = make_in_maps(x, wq, bq, wk, bk, wv, bv, wo, bo)
    res = run_sharded(nc, in_maps)
    return gather(res.results, host_bias(bv, wo, bo))

